# revision 2
# baseline (speedup 1.0000x reference)
"""ColorCurveLearningLoss on 8 Trainium2 NeuronCores (~112 us/core, was 182).

Factorized 4(hi) x 8(lo) one-hot histogram via TensorE matmul.

Math: pred_curve - target_curve needs per-(channel,bin) sums of d = p - t
and counts. bins = 32 = 4 hi * 8 lo. Per element:
  idx  = floor(32x) computed on ScalarE via fp32 magic rounding:
         y = 32x + 0.5 (exact); v = y + 2^23 (RNE -> 2^23 + floor(32x) + 1);
         idxb = v - (2^23+1)  -> exact bin index as bf16 value (x=0 -> -1).
  lo one-hot: th[j] = is_equal(mod(idxb, 8), j)   (8 chained ts ops, 4x mode)
  hi via cumulative masks g_a = is_ge(idxb, 8a), a=1..3.
  lhsT planes L = [d, d*g1, d*g2, d*g3, 1, g1, g2, g3] (plane-major).
  matmul(L[:, :, 16cols], th[:, :, 16cols]) -> PSUM[128,128] accumulated
  over the whole channel; host decodes diag, differences the cumsums.
d is bf16 (p,t loaded via SWDGE cast-DMA); sums accumulate in fp32 PSUM.
"""

import numpy as np

NB = 32
B, C, H, W = 16, 3, 512, 512
N_CORES = 8
B_PER_CORE = B // N_CORES
ELEMS_PER_CH = B_PER_CORE * H * W  # 524288
P = 128
F = 1024
N_SUPER = ELEMS_PER_CH // (P * F)  # 4
GROUP = 16
NG = F // GROUP  # 64 matmuls per sub

USE_BF16_DMA = True

_CACHE = {}

_B05 = 0.5
_B2P23 = float(2.0 ** 23)
_BM = -float(2.0 ** 23 + 1.0)


def _build():
    import concourse.bass as bass
    import concourse.tile as tile
    from concourse import bacc, mybir

    nc = bacc.Bacc("TRN2", target_bir_lowering=False, debug=False,
                   num_devices=N_CORES)
    f32 = mybir.dt.float32
    bf16 = mybir.dt.bfloat16
    Identity = mybir.ActivationFunctionType.Identity
    Copy = mybir.ActivationFunctionType.Copy
    Alu = mybir.AluOpType

    for val in (_B05, _B2P23, _BM, 1.0):
        t = nc.alloc_sbuf_tensor(f"constx-{val}", [128, 1], f32)
        nc.gpsimd.memset(t.ap(), val)
        nc.const_aps.aps[(f32, val)] = t.ap()
    nc.all_engine_barrier()

    in_dt = f32
    xin = nc.dram_tensor("xin", [C, N_SUPER, P, NG, GROUP], f32,
                         kind="ExternalInput")
    pin = nc.dram_tensor("pin", [C, N_SUPER, P, NG, GROUP], in_dt,
                         kind="ExternalInput")
    tin = nc.dram_tensor("tin", [C, N_SUPER, P, NG, GROUP], in_dt,
                         kind="ExternalInput")
    out = nc.dram_tensor("out", [P, C * 128], f32,
                         kind="ExternalOutput")

    pt_dt = bf16 if USE_BF16_DMA else f32

    with tile.TileContext(nc) as tc:
        with (
            tc.tile_pool(name="inp", bufs=3) as inp,
            tc.tile_pool(name="work", bufs=2) as work,
            tc.tile_pool(name="acc", bufs=1) as accp,
            tc.tile_pool(name="ps", bufs=1, space="PSUM") as ps,
        ):
            psum = ps.tile([P, C, 512], f32, tag="psum", name="psum")
            res = accp.tile([P, C * 128], f32, name="res")

            for c in range(C):
                n_mm = N_SUPER * NG
                mm_i = 0
                for sb in range(N_SUPER):
                    xt = inp.tile([P, NG, GROUP], f32, tag="x", name="xt")
                    nc.sync.dma_start(out=xt[:], in_=xin[c, sb])
                    pt = inp.tile([P, NG, GROUP], pt_dt, tag="p", name="pt")
                    tt_in = inp.tile([P, NG, GROUP], pt_dt, tag="t", name="tt")
                    if USE_BF16_DMA:
                        nc.gpsimd.dma_start(out=pt[:], in_=pin[c, sb])
                        nc.gpsimd.dma_start(out=tt_in[:], in_=tin[c, sb])
                    else:
                        nc.sync.dma_start(out=pt[:], in_=pin[c, sb])
                        nc.sync.dma_start(out=tt_in[:], in_=tin[c, sb])

                    # ScalarE floor pipeline (see module docstring)
                    yv = work.tile([P, NG, GROUP], f32, tag="yv", name="yv")
                    nc.scalar.activation(out=yv[:], in_=xt[:], func=Identity,
                                         bias=_B05, scale=float(NB))
                    nc.scalar.activation(out=yv[:], in_=yv[:], func=Identity,
                                         bias=_B2P23, scale=1.0)
                    # idxb grouped [P, NG, GROUP] (contiguous == [P, F])
                    idxb = work.tile([P, NG, GROUP], bf16, tag="idxb",
                                     name="idxb")
                    nc.scalar.activation(out=idxb[:], in_=yv[:],
                                         func=Identity, bias=_BM, scale=1.0)

                    # lo one-hot. yv holds v = 2^23 + idx + 1 (fp32), so
                    # mantissa low bits of v are (idx+1); idx%8 == j iff
                    # (v_int & 7) == (j+1)%8.
                    wi = yv[:].bitcast(mybir.dt.int32)
                    li = work.tile([P, NG, GROUP], mybir.dt.int32, tag="li",
                                   name="li")
                    nc.vector.tensor_scalar(
                        out=li[:], in0=wi, scalar1=7, scalar2=None,
                        op0=Alu.bitwise_and)
                    lob = work.tile([P, NG, GROUP], bf16, tag="lob",
                                    name="lob")
                    nc.scalar.activation(out=lob[:], in_=li[:], func=Copy)
                    th = work.tile([P, 8, NG, GROUP], bf16, tag="th",
                                   name="th")
                    for j in range(8):
                        nc.vector.tensor_scalar(
                            out=th[:, j, :, :], in0=lob[:],
                            scalar1=float((j + 1) % 8), scalar2=None,
                            op0=Alu.is_equal)

                    # lhsT group-blocked: [P, NG, 8*GROUP]; group gi block =
                    # planes [d, d*g1, d*g2, d*g3, ones, g1, g2, g3] x 16 cols
                    # (contiguous per group -> legal 1-free-dim matmul AP).
                    L = work.tile([P, NG, 8 * GROUP], bf16, tag="L", name="L")

                    def pl(a):
                        return L[:, :, a * GROUP:(a + 1) * GROUP]

                    nc.vector.tensor_scalar(
                        out=pl(4), in0=idxb[:],
                        scalar1=0.0, scalar2=1.0,
                        op0=Alu.mult, op1=Alu.add)
                    for a in (1, 2, 3):
                        nc.vector.tensor_scalar(
                            out=pl(4 + a), in0=idxb[:],
                            scalar1=float(8 * a), scalar2=None,
                            op0=Alu.is_ge)
                    nc.vector.tensor_tensor(
                        out=pl(0), in0=pt[:], in1=tt_in[:],
                        op=Alu.subtract)
                    for a in (1, 2, 3):
                        nc.vector.tensor_tensor(
                            out=pl(a), in0=pl(0), in1=pl(4 + a),
                            op=Alu.mult)

                    for gi in range(NG):
                        nc.tensor.matmul(
                            psum[:, c, 0:128],
                            lhsT=L[:, gi, :],
                            rhs=th[:, :, gi, :],
                            start=(mm_i == 0),
                            stop=(mm_i == n_mm - 1),
                        )
                        mm_i += 1

                nc.scalar.copy(out=res[:, c * 128:(c + 1) * 128],
                               in_=psum[:, c, 0:128])

            nc.sync.dma_start(out=out[:], in_=res[:])

    nc.compile()
    return nc


def _get_nc():
    if "nc" not in _CACHE:
        _CACHE["nc"] = _build()
    return _CACHE["nc"]


def _shard(arr, core):
    a = arr[core * B_PER_CORE:(core + 1) * B_PER_CORE]
    a = np.ascontiguousarray(np.transpose(a, (1, 0, 2, 3)))
    return a.reshape(C, N_SUPER, P, NG, GROUP).astype(np.float32, copy=False)


def _decode(raw):
    """raw [P, C*128] -> cumulative S rows/C rows -> per-bin S, Cnt."""
    S = np.zeros((C, NB), np.float64)
    Cnt = np.zeros((C, NB), np.float64)
    for c in range(C):
        Pm = raw[:, c * 128:(c + 1) * 128].astype(np.float64)
        R = Pm.reshape(8, GROUP, 8, GROUP)
        cum = np.einsum('pglg->pl', R)  # [plane, lo]
        ds = cum[0:4]   # d cumulative over hi
        cs = cum[4:8]   # count cumulative over hi
        for hi in range(4):
            up_d = ds[hi + 1] if hi < 3 else 0.0
            up_c = cs[hi + 1] if hi < 3 else 0.0
            S[c, 8 * hi:8 * hi + 8] = ds[hi] - up_d
            Cnt[c, 8 * hi:8 * hi + 8] = cs[hi] - up_c
    return S, Cnt


def _finalize(S, Cnt):
    diff = np.where(Cnt > 0, np.abs(S) / np.maximum(Cnt, 1.0), 0.0)
    return np.float32(diff.mean())


def kernel(pred, target, input_img):
    from concourse.bass_utils import run_bass_kernel_spmd

    nc = _get_nc()
    in_maps = []
    for core in range(N_CORES):
        in_maps.append({
            "xin": _shard(np.asarray(input_img), core),
            "pin": _shard(np.asarray(pred), core),
            "tin": _shard(np.asarray(target), core),
        })
    res = run_bass_kernel_spmd(nc, in_maps, list(range(N_CORES)))
    S = np.zeros((C, NB), np.float64)
    Cnt = np.zeros((C, NB), np.float64)
    for r in res.results:
        s, cc = _decode(r["out"])
        S += s
        Cnt += cc
    _CACHE["last_SC"] = (S, Cnt)
    return np.asarray(_finalize(S, Cnt), dtype=np.float32)
